# revision 1
# baseline (speedup 1.0000x reference)
"""Trainium2 Bass kernel for nn_Attention (GQA with group-summed query heads).

Algorithm notes (validated against reference in numpy):
- The reference einsum 'bghnd,bhsd->bhns' SUMS over the query-group axis, so the
  16 query heads collapse into 4 effective heads: wq columns can be pre-summed
  per kv-head (RoPE is linear per-position, both /sqrt(64) scalings folded in).
- This makes the problem plain 4-head attention: 2 batches x 4 kv-heads = 8
  independent (b,h) attention instances -> one per NeuronCore.
- Head dims are pair-permuted [t1(even), t2(odd)] so RoPE becomes wide
  elementwise multiply-adds: P1 = W1.T@xT (q/k stacked on 32-row blocks),
  P2 = signed pair-swap of P1 done by ONE permutation-matrix matmul, then
  rot = P1*[c;c;c;c] + P2*[s;s;s;s].
- Scores are computed TRANSPOSED (scoresT[key, query]) so exp(scoresT) is
  directly usable as the AV matmul's stationary-side rhs with V as lhsT; an
  all-ones column appended to V yields the softmax denominators for free.
- No max-subtraction needed: scores = q_eff . k / 64 with |scores| <~ 1.5.
- Softmax denominators come free via an all-ones column appended to V; the
  normalization happens BEFORE the AllToAll (reciprocal of the sums row +
  ones-outer-product matmul broadcast), so the collective carries bf16 [64,512]
  blocks only.
- Final: 8-core AllToAll redistributes per-head outputs into 512-row output
  slices; each core applies the row-parallel wo matmul with head-pairs stacked
  to K=128.
"""

import sys
import os

sys.path.insert(0, "/opt/trn_rl_repo")

import numpy as np
import ml_dtypes

B, S, D = 2, 4096, 1024
QH, KVH, HD = 16, 4, 64
KV_DIM = KVH * HD  # 256
NCORES = 8
SB = 512           # s-block / q-block width
NSB = S // SB      # 8
GRP = 2            # key-chunks (128 keys each) per exp group -> [128, 1024] psum

_CACHE = {}


def _build_nc(stop_after="D", collective=True):
    import concourse.bacc as bacc
    import concourse.tile as tile
    from concourse import mybir

    f32 = mybir.dt.float32
    bf = mybir.dt.bfloat16
    f8 = mybir.dt.float8e4
    EXP = mybir.ActivationFunctionType.Exp

    nc = bacc.Bacc("TRN2", target_bir_lowering=False, debug=False,
                   num_devices=NCORES)

    xT_d = nc.dram_tensor("xT", [D, S], bf, kind="ExternalInput")
    w1_d = nc.dram_tensor("w1", [D, 128], f32, kind="ExternalInput")
    wv_d = nc.dram_tensor("wv", [D, HD], f32, kind="ExternalInput")
    ab_d = nc.dram_tensor("ab", [2, 128, S], bf, kind="ExternalInput")
    mk_d = nc.dram_tensor("masks", [4, 128, SB], bf, kind="ExternalInput")
    wo_d = nc.dram_tensor("wo", [KV_DIM, D], f32, kind="ExternalInput")
    out_d = nc.dram_tensor("out", [B, SB, D], f32, kind="ExternalOutput")

    idn_d = nc.inline_tensor(np.eye(HD, dtype=np.float32), "idn")
    Mperm = np.zeros((128, 128), np.float32)
    for r in range(0, 32):
        Mperm[r, r + 32] = -1.0
        Mperm[r + 32, r] = 1.0
        Mperm[r + 64, r + 96] = -1.0
        Mperm[r + 96, r + 64] = 1.0
    permT_d = nc.inline_tensor(Mperm.T.astype(ml_dtypes.bfloat16), "permT")

    with tile.TileContext(nc) as tc:
        with (
            tc.tile_pool(name="persist", bufs=1) as pp,
            tc.tile_pool(name="work", bufs=4) as wp,
            tc.tile_pool(name="expp", bufs=4) as ep,
            tc.tile_pool(name="ps_sc", bufs=2, space="PSUM") as ps_sc,
            tc.tile_pool(name="ps_p", bufs=1, space="PSUM") as ps_p,
            tc.tile_pool(name="ps_aux", bufs=1, space="PSUM") as ps_aux,
            tc.tile_pool(name="dram", bufs=1, space="DRAM") as dp,
        ):
            # ---- persistent SBUF tensors ----
            xT = pp.tile([128, 8 * S], bf, tag="xT")          # 64KB/part
            w1 = pp.tile([128, 8 * 128], bf, tag="w1")
            wv = pp.tile([128, 8 * HD], bf, tag="wv")
            At = pp.tile([128, S], bf, tag="At")
            Bt = pp.tile([128, S], bf, tag="Bt")
            mk = pp.tile([128, 4 * SB], bf, tag="mk")
            wo = pp.tile([128, 2 * D], bf, tag="wo")          # head-pair c rows 128c
            rot = pp.tile([128, S], bf, tag="rot")            # rows 0:64 q~, 64:128 k~
            rotk = pp.tile([HD, S], bf, tag="rotk")           # k~ at base partition 0
            rotq2 = pp.tile([128, S], bf, tag="rotq2")        # q~ dup at rows 64:128
            VS = HD + 1
            vaug = pp.tile([128, 32 * VS], bf, tag="vaug")
            outTs = []
            for q in range(NSB):
                oT = pp.tile([HD, SB], bf, tag=f"outT{q}")
                outTs.append(oT)
            idn = pp.tile([HD, HD], f32, tag="idn")
            permT = pp.tile([128, 128], bf, tag="permT")
            ones65 = pp.tile([HD + 1, HD], f32, tag="ones65")

            # ---- input loads (gpsimd = SWDGE casts f32->bf16 in flight) ----
            # weights first (small, needed by the first projection)
            w1_3 = w1[:, :].rearrange("p (c m) -> p c m", c=8)
            w1d3 = w1_d[:, :].rearrange("(c p) m -> p c m", p=128)
            nc.gpsimd.dma_start(w1_3[:, :, :], w1d3[:, :, :])
            # first x s-block right after w1/w2 so P(0) starts ASAP
            xT3 = xT[:, :].rearrange("p (c s) -> p c s", c=8)
            xTd3 = xT_d[:, :].rearrange("(c p) s -> p c s", p=128)
            nc.gpsimd.dma_start(xT3[:, :, 0: SB], xTd3[:, :, 0: SB])
            wv_3 = wv[:, :].rearrange("p (c m) -> p c m", c=8)
            wvd3 = wv_d[:, :].rearrange("(c p) m -> p c m", p=128)
            nc.gpsimd.dma_start(wv_3[:, :, :], wvd3[:, :, :])
            wo_3 = wo[:, :].rearrange("p (c n) -> p c n", c=2)
            wod3 = wo_d[:, :].rearrange("(c p) n -> p c n", p=128)
            nc.gpsimd.dma_start(wo_3[:, :, :], wod3[:, :, :])
            nc.sync.dma_start(At[:, :], ab_d[0, :, :])
            nc.sync.dma_start(Bt[:, :], ab_d[1, :, :])
            mk3 = mk[:, :].rearrange("p (r m) -> p r m", r=4)
            nc.sync.dma_start(mk3[:, :, :], mk_d[:, :, :].rearrange("r p m -> p r m"))
            nc.sync.dma_start(idn[:, :], idn_d[:, :])
            nc.sync.dma_start(permT[:, :], permT_d[:, :])
            nc.vector.memset(ones65[:, :], 1.0)
            for c in range(32):
                nc.vector.memset(vaug[:, VS * c + HD: VS * c + HD + 1], 1.0)
            # s-sliced x loads: one DMA brings ALL 8 D-chunks for one s-block,
            # so the s-block-j projection starts after load j (not after all 8)
            for j in range(1, NSB):
                nc.gpsimd.dma_start(xT3[:, :, SB * j: SB * (j + 1)],
                                    xTd3[:, :, SB * j: SB * (j + 1)])

            lvl = ["L", "P", "A", "C", "D"].index(stop_after)
            if lvl < 1:
                nc.gpsimd.dma_start(out_d[0, 0:128, 0:4096//4], xT[:, 0:1024])
            if lvl >= 3:
                bin_ = dp.tile([NCORES, HD, 512], bf, tag="bin")
                bout = dp.tile([NCORES, HD, 512], bf, tag="bout")

            # ---- stages P+A interleaved per s-block ----
            # P(j): projections + RoPE + V-transpose for s-block j.
            # A(qb=j): full attention row-band for q-block j (needs rot/v of
            # s-blocks 0..j only, all available after P(j)).
            for j in range(NSB if lvl >= 1 else 0):
                # ---- P(j) ----
                sc = ps_p.tile([128, 3 * SB], f32, tag="pp")

                def xs(d8, _j=j):
                    return xT[:, S * d8 + SB * _j: S * d8 + SB * (_j + 1)]

                for d8 in range(8):
                    nc.tensor.matmul(sc[:, 0:512], w1[:, 128 * d8: 128 * (d8 + 1)],
                                     xs(d8), start=(d8 == 0), stop=(d8 == 7))
                for d8 in range(8):
                    nc.tensor.matmul(sc[0:HD, 1024:1536], wv[:, HD * d8: HD * (d8 + 1)],
                                     xs(d8), start=(d8 == 0), stop=(d8 == 7))

                # P2 = signed pair-swap of P1 -> ONE permutation matmul
                p1s = wp.tile([128, SB], bf, tag="p1s")
                nc.vector.tensor_copy(p1s[:, :], sc[:, 0:512])
                nc.tensor.matmul(sc[:, 512:1024], permT[:, :], p1s[:, :],
                                 start=True, stop=True)

                u = wp.tile([128, SB], bf, tag="u")
                w_ = wp.tile([128, SB], f32, tag="w_")
                nc.vector.tensor_mul(u[:, :], p1s[:, :], At[:, SB * j: SB * (j + 1)])
                nc.vector.tensor_mul(w_[:, :], sc[:, 512:1024], Bt[:, SB * j: SB * (j + 1)])
                nc.vector.tensor_add(rot[:, SB * j: SB * (j + 1)], u[:, :], w_[:, :])
                # k~ copy down to base partition 0 (DMA moves across partitions)
                nc.gpsimd.dma_start(rotk[:, SB * j: SB * (j + 1)],
                                    rot[64:128, SB * j: SB * (j + 1)])
                # q~ copy up to partitions 64:128 so QK^T can 2-way row-pack
                nc.gpsimd.dma_start(rotq2[64:128, SB * j: SB * (j + 1)],
                                    rot[0:64, SB * j: SB * (j + 1)])

                vts = wp.tile([HD, SB], f32, tag="vts")
                nc.vector.tensor_copy(vts[:, :], sc[0:HD, 1024:1536])
                # transposes reuse this j's pp bank 0 (P1 already consumed by rope)
                for t in range(4):
                    nc.tensor.transpose(sc[:, HD * t: HD * (t + 1)],
                                        vts[:, 128 * t: 128 * (t + 1)], idn[:, :])
                for t in range(4):
                    cch = 4 * j + t
                    nc.vector.tensor_copy(vaug[:, VS * cch: VS * cch + HD],
                                           sc[:, HD * t: HD * (t + 1)])

                if lvl < 2:
                    continue
                # ---- A(qb=j) ----
                qb = j
                po = ps_aux.tile([HD + 1, 512], f32, tag="po")
                nk = 4 * (qb + 1)
                for g0 in range(0, nk, GRP):
                    cnt = min(GRP, nk - g0)
                    sc = ps_sc.tile([128, GRP * SB], f32, tag="sc")
                    for r in range(cnt):
                        kb = g0 + r
                        dst = sc[:, 512 * r: 512 * (r + 1)]
                        if r % 2 == 0:
                            # row-tile T0: k~/q~ from partitions 0:64
                            nc.tensor.matmul(dst, rotk[:, 128 * kb: 128 * (kb + 1)],
                                             rot[0:HD, SB * qb: SB * (qb + 1)],
                                             start=True, stop=True)
                        else:
                            # row-tile T8: concurrent with the T0 matmul
                            nc.tensor.matmul(dst, rot[64:128, 128 * kb: 128 * (kb + 1)],
                                             rotq2[64:128, SB * qb: SB * (qb + 1)],
                                             start=True, stop=True)
                    pe = ep.tile([128, GRP * SB], bf, tag="pe")
                    nc.scalar.activation(pe[:, 0: 512 * cnt], sc[:, 0: 512 * cnt], EXP)
                    for r in range(cnt):
                        di = (g0 + r) - (nk - 4)
                        if di >= 0:
                            mw = 128 * (di + 1)  # all-ones beyond this col
                            nc.vector.tensor_mul(pe[:, 512 * r: 512 * r + mw],
                                                 pe[:, 512 * r: 512 * r + mw],
                                                 mk[:, SB * di: SB * di + mw])
                    for r in range(cnt):
                        kb = g0 + r
                        nc.tensor.matmul(po[:, :],
                                         vaug[:, VS * kb: VS * kb + HD + 1],
                                         pe[:, 512 * r: 512 * (r + 1)],
                                         start=(kb == 0), stop=(kb == nk - 1))
                # normalize locally BEFORE the a2a: recip of the sums row
                # (partition 64) into SBUF, matmul-broadcast to 64 partitions
                rcs = wp.tile([HD + 1, 512], f32, tag="rcs")
                nc.vector.reciprocal(rcs[HD:HD + 1, :], po[HD:HD + 1, :])
                bc = ps_sc.tile([HD, 512], f32, tag="sc")
                nc.tensor.matmul(bc[:, :], ones65[HD:HD + 1, :], rcs[HD:HD + 1, :],
                                 start=True, stop=True)
                # walrus: a DVE op may read at most ONE PSUM operand -> stage po
                nc.any.tensor_copy(outTs[qb][:, :], po[0:HD, :])
                nc.vector.tensor_mul(outTs[qb][:, :], outTs[qb][:, :], bc[:, :])
                if lvl >= 3:
                    # eager bounce-out: slice qb of outT is exactly a2a block qb
                    nc.sync.dma_start(bin_[qb, :, :], outTs[qb][:, :])

            # ---- stage C: AllToAll (512-col output slices across all 8 cores) ----
            if lvl == 1:
                nc.gpsimd.dma_start(out_d[0, 0:128, 0:1024], rot[:, 0:1024])
                nc.gpsimd.dma_start(out_d[1, 0:128, 0:1024], vaug[:, 0:1024])
            if lvl == 2:
                nc.gpsimd.dma_start(out_d[0, 0:64, 0:512], outTs[0][:, :])
                nc.gpsimd.dma_start(out_d[0, 0:64, 512:1024], outTs[1][:, :])
            if lvl >= 3:
                if collective:
                    from concourse import mybir as _mb
                    nc.gpsimd.collective_compute(
                        "AllToAll", _mb.AluOpType.bypass,
                        replica_groups=[list(range(NCORES))],
                        ins=[bin_.opt()], outs=[bout.opt()],
                    )
                else:
                    # single-core timeline-sim stand-in: local DRAM->DRAM move
                    nc.sync.dma_start(bout[:, :, :], bin_[:, :, :])
                if lvl == 3:
                    nc.sync.dma_start(out_d[0, 0:65, 0:512], bout[0, :, :])

            # ---- stage D: wo matmul + output (inputs already normalized) ----
            if lvl >= 4:
                # head pairs stacked on partition halves -> K=128 wo matmuls
                g2 = pp.tile([128, 4 * 512], bf, tag="g2")
                g2lo = g2[0:HD, :].rearrange("p (m s) -> p m s", m=4)
                g2hi = g2[HD:128, :].rearrange("p (m s) -> p m s", m=4)
                bt3 = bout[:, :, :].rearrange("(m e) p s -> e p m s", e=2)
                nc.sync.dma_start(g2lo[:, :, :], bt3[0, :, :, :])
                nc.scalar.dma_start(g2hi[:, :, :], bt3[1, :, :, :])
                for b in range(B):
                    for t in range(4):
                        ys = wp.tile([128, 1024], f32, tag="ys")
                        for nh in range(2):
                            # stage-A sc slots are free here; reuse for 2x
                            # double-buffered wo-psum
                            yp = ps_sc.tile([128, 512], f32, tag="sc")
                            for pr in range(2):
                                m = 2 * b + pr
                                nc.tensor.matmul(yp[:, :],
                                                 g2[:, 512 * m + 128 * t: 512 * m + 128 * (t + 1)],
                                                 wo[:, D * pr + 512 * nh: D * pr + 512 * (nh + 1)],
                                                 start=(pr == 0), stop=(pr == 1))
                            nc.any.tensor_copy(ys[:, 512 * nh: 512 * (nh + 1)], yp[:, :])
                        eng = (nc.sync, nc.scalar, nc.gpsimd)[(4 * b + t) % 3]
                        eng.dma_start(out_d[b, 128 * t: 128 * (t + 1), :], ys[:, :])

    nc.compile()
    return nc


def _get_nc():
    if "nc" not in _CACHE:
        _CACHE["nc"] = _build_nc()
    return _CACHE["nc"]


def _prep_in_maps(x, wq, wk, wv, wo, freq_cos, freq_sin):
    x = np.asarray(x, np.float32)
    wq = np.asarray(wq, np.float32)
    wk = np.asarray(wk, np.float32)
    wv = np.asarray(wv, np.float32)
    wo = np.asarray(wo, np.float32)
    cos = np.asarray(freq_cos, np.float32)
    sin = np.asarray(freq_sin, np.float32)

    # group-sum wq per kv head (einsum sums over group axis); fold both /8 scales
    wqr = wq.reshape(D, QH, HD)
    wq_eff = np.stack([wqr[:, h::KVH].sum(axis=1) for h in range(KVH)], axis=1) / 64.0
    wkr = wk.reshape(D, KVH, HD)
    W1 = np.empty((KVH, D, 128), np.float32)
    for h in range(KVH):
        q1, q2 = wq_eff[:, h, 0::2], wq_eff[:, h, 1::2]
        k1, k2 = wkr[:, h, 0::2], wkr[:, h, 1::2]
        W1[h] = np.concatenate([q1, q2, k1, k2], axis=1)
    Wv = np.ascontiguousarray(wv.reshape(D, KVH, HD).transpose(1, 0, 2))

    A = np.tile(cos.T, (4, 1)).astype(ml_dtypes.bfloat16)   # [128, S]
    Bm = np.tile(sin.T, (4, 1)).astype(ml_dtypes.bfloat16)
    ab = np.ascontiguousarray(np.stack([A, Bm]))

    qi = np.arange(SB)[None, :]
    ki = np.arange(128)[:, None]
    masks = np.ascontiguousarray(
        np.stack([(qi >= ki + 128 * r) for r in range(4)]).astype(ml_dtypes.bfloat16))

    xTb = [np.ascontiguousarray(x[b].T).astype(ml_dtypes.bfloat16) for b in range(B)]

    in_maps = []
    for c in range(NCORES):
        b, h = c // KVH, c % KVH
        in_maps.append({
            "xT": xTb[b],
            "w1": np.ascontiguousarray(W1[h]),
            "wv": Wv[h],
            "ab": ab,
            "masks": masks,
            "wo": wo,
        })
    return in_maps


def _assemble(results):
    full = np.empty((B, S, D), np.float32)
    for c in range(NCORES):
        y = results[c]["out"]  # [B, 512, D]
        for b in range(B):
            full[b, SB * c: SB * (c + 1), :] = y[b]
    return full


def _ensure_axon_hooks_stub():
    # slim axon builds lack antenv.axon_hooks; degrade trace=True gracefully
    try:
        import antenv.axon_hooks  # noqa: F401
    except Exception:
        import types
        m = types.ModuleType("antenv.axon_hooks")
        m.get_axon_ntff_profile_hook = lambda: None
        sys.modules["antenv.axon_hooks"] = m


def run(in_maps, trace=False):
    from concourse.bass_utils import run_bass_kernel_spmd
    _ensure_axon_hooks_stub()
    nc = _get_nc()
    res = run_bass_kernel_spmd(nc, in_maps, core_ids=list(range(NCORES)),
                               trace=trace)
    return res


def kernel(**inputs):
    in_maps = _prep_in_maps(**inputs)
    res = run(in_maps, trace=False)
    return _assemble(res.results)


if __name__ == "__main__":
    # smoke: build only
    _get_nc()
    print("built ok")

